# revision 43
# baseline (speedup 1.0000x reference)
"""Multi-head attention (B=2, S=2048, D=1024, H=16) on 8 TRN2 NeuronCores.

Sharding: core c handles batch c//4 and heads 4*(c%4) .. 4*(c%4)+4
(tensor-parallel over heads x data-parallel over batch).

Per-core pipeline (fp16 matmuls, fp32 PSUM):
  1. qT/kT = W @ X^T  [d=256 on partitions, s free]; v = X @ W^T [s, d] with
     a ones column appended per head (softmax denominator rides the A*V).
  2. Global 128-step software pipeline over 8 (qs, pair) blocks x 16 k-blocks:
       PE:  S^T[k,q] = kT.T @ qT  (row-packed head pair)       -> sc PSUM
       ACT: p = exp(S/8)                                       -> pt SBUF
       PE:  att[q, d|den] += pt.T @ [v|1]   (pt stationary, v moving: the
            moving stream is 65 columns instead of 512)
     Block end: per-q reciprocal of the den column, broadcast-multiply
     (DVE), then 4 DMA-xbar transposes att[q,d] -> outt[d,q]; the last
     block transposes on the then-idle PE (is_transpose+identity).
  3. partial[s, 1024] = outt.T @ WoT, interleaved into the PE slack of the
     last blocks + a small tail; one merged [128, 1024] DMA per s-block.
Host: full output[b] = sum of the 4 partials for batch b + b_o.

Scheduling notes (cost-model driven):
  - The scalar engine's 128 exp instructions (~1038ns each) are the pacing
    stream; total ~= first-exp + 128*exp + last-block tail.  The DMA feed
    is ordered so the first score block's inputs (wq half, xq s0:512,
    wk half, xk s0:256) land first; weights are pre-split into
    partition-major dc-halves so a half transfers at full DMA rate.
  - The PE is kept continuously busy from t~0.5us with dummy warmup
    matmuls so the cost model's p-state ramp is complete before real work,
    and real work starts as soon as the first transfer lands.
  - Block order puts pair-1 of qs3 in the middle so only one O-projection
    (qs2) trails the pipeline; its norm/transpose/oproj chain is pipelined
    per 128-q group.
  - Projection/O-proj chains are split into ~430ns pieces and drained from
    a deadline-ordered queue into each pipeline step, alternating between
    two single-bank PSUM tags.
"""
from collections import deque

import numpy as np

import concourse.bass as bass
import concourse.mybir as mybir
from concourse.tile import TileContext
from concourse.bass_utils import run_bass_kernel_spmd


def split_multi_waits(nc):
    """This container's walrus codegen allows only one sync-wait command per
    instruction ("Too many sync wait commands" in setupSyncWait). Tile
    sometimes attaches several semaphore waits to one instruction; hoist the
    extras onto dedicated EventSemaphore instructions inserted immediately
    before, on the same engine (sequencers execute in order, so semantics
    are identical)."""
    n = [0]
    for f in nc.m.functions:
        for blk in f.blocks:
            new_insts = []
            changed = False
            for inst in blk.instructions:
                si = inst.sync_info
                if si is not None and len(si.on_wait) > 1:
                    waits = list(si.on_wait)
                    for w in waits[:-1]:
                        n[0] += 1
                        ev = mybir.InstEventSemaphore(
                            name=f"WSPLIT-{n[0]}",
                            ins=[], outs=[],
                        )
                        ev.engine = inst.engine
                        ev.sync_info = mybir.SyncInfo(on_wait=[w], on_update=[])
                        new_insts.append(ev)
                        nc.register_instruction(ev, overwrite=True)
                    inst.sync_info = mybir.SyncInfo(
                        on_wait=[waits[-1]], on_update=list(si.on_update)
                    )
                    changed = True
                new_insts.append(inst)
            if changed:
                blk.instructions = new_insts
    return n[0]


F16 = mybir.dt.float16
F32 = mybir.dt.float32

B, S, D = 2, 2048, 1024
H, DK = 16, 64
HPC = 4              # heads per core
DC = HPC * DK        # 256 d-model dims per core
N_CORES = 8
P = 128              # partitions
FC = D // P          # 8 feature chunks (contraction for projections)
KB = S // P          # 16 k-blocks
QSUP = 512           # q tile width per block
NQ = S // QSUP       # 4 q supertiles
NBLK = 2 * NQ        # 8 (qs, pair) blocks
NSTEP = NBLK * KB    # 128 pipeline steps
LAG = 16             # max av lag (relaxes v-proj deadlines vs the DMA feed)
# per-block av lag: early blocks need the full lag for the serial v feed;
# later blocks shrink by 1 per block (the av psum double-buffer still closes:
# block b's norm precedes block b+1's first-av allocation) so the last block
# trails the exp stream by only 9 steps
LAGS = [LAG] * NBLK
AV_KB0_DELAY = True
TEXP0 = 11000
DRAIN_EARLY = 1
STEPNS = 1100
# drain-queue deadline tuning knobs (exp-step units)
V0_DL = LAG - 1       # v(sb, hh0) deadline offset (+sb)
QK_DL = -6            # qk chain deadline: 16*ss + QK_DL + 3*sh
K1_DL = 58            # kcol(1, kb) deadline offset (+kb)
Q1_DL = 52            # pair-1 qk chains: Q1_DL + 4*i + 2*sh
V1_DL = 77            # v(sb, hh1) deadline offset (+sb)
OP_SPREAD = 3         # oproj piece deadline spacing per sb

# block order: pair-0 in qs order, then pair-1 with qs3 first, so the qs3
# O-projection (needing both pairs) runs mid-pipeline and only qs2's trails
BLOCKS = [(0, 0), (1, 0), (2, 0), (3, 0), (3, 1), (0, 1), (1, 1), (2, 1)]
# exp-step at which each qs has both pairs' normed att available (+4 steps
# for the norm + DMA-transpose latency; qs1's is late enough that its pieces
# drain into the trailing-av chase)
OPROJ_READY = {3: 98, 0: 116, 1: 132}


def build_bass():
    nc = bass.Bass()
    xtq = nc.dram_tensor("xtq", [D, S], F16, kind="ExternalInput")
    xtk = nc.dram_tensor("xtk", [D, S], F16, kind="ExternalInput")
    xtv = nc.dram_tensor("xtv", [D, S], F16, kind="ExternalInput")
    # weights pre-split into partition-major dc-halves: [P, FC*P] with the
    # (fc, d) pairs contiguous per partition -> full-rate 2KB DMA elements
    wq_h = [nc.dram_tensor(f"wqt{h}", [P, FC * P], F16, kind="ExternalInput")
            for h in range(2)]
    wk_h = [nc.dram_tensor(f"wkt{h}", [P, FC * P], F16, kind="ExternalInput")
            for h in range(2)]
    wv_h = [nc.dram_tensor(f"wvt{h}", [P, FC * P], F16, kind="ExternalInput")
            for h in range(2)]
    wot = nc.dram_tensor("wot", [DC, D], F16, kind="ExternalInput")
    bq = nc.dram_tensor("bq", [DC, 1], F32, kind="ExternalInput")
    bk = nc.dram_tensor("bk", [DC, 1], F32, kind="ExternalInput")
    bvr = nc.dram_tensor("bvr", [1, DC], F32, kind="ExternalInput")
    ident = nc.dram_tensor("ident", [P, P], F16, kind="ExternalInput")
    outp = nc.dram_tensor("outp", [S, D], F16, kind="ExternalOutput")

    with TileContext(nc) as tc:
        consts = tc.alloc_tile_pool(name="consts", bufs=1)
        qkv = tc.alloc_tile_pool(name="qkv", bufs=1)
        ptpool = tc.alloc_tile_pool(name="ptpool", bufs=LAG + 2)
        attpool = tc.alloc_tile_pool(name="attpool", bufs=3)
        dyn = tc.alloc_tile_pool(name="dyn", bufs=2)
        opool = tc.alloc_tile_pool(name="opool", bufs=4)
        # PSUM banks: sc 2x2 + av 2x1 + proj ps0/ps1 1x1 each = 8
        sc_pool = tc.alloc_tile_pool(name="sc_pool", bufs=2, space="PSUM")
        av_pool = tc.alloc_tile_pool(name="av_pool", bufs=2, space="PSUM")
        psp = tc.alloc_tile_pool(name="psp", bufs=1, space="PSUM")

        # ---- SBUF tiles ----
        # weights laid out dc-chunk-major: [P, dc_half, FC, P]
        wq_sb = consts.tile([P, 2, FC, P], F16, tag="wq")
        wk_sb = consts.tile([P, 2, FC, P], F16, tag="wk")
        wv_sb = consts.tile([P, 2, FC, P], F16, tag="wv")
        wo_sb = consts.tile([P, 2, D], F16, tag="wo")
        bq_sb = consts.tile([P, 2, 1], F32, tag="bq")
        bk_sb = consts.tile([P, 2, 1], F32, tag="bk")
        bv_row = consts.tile([1, DC], F32, tag="bvrow")
        xk_sb = consts.tile([P, FC, S], F16, tag="xk")
        xq_sb = consts.tile([P, FC, S], F16, tag="xq")
        xv_sb = consts.tile([P, FC, S], F16, tag="xv")
        ident_sb = consts.tile([P, P], F16, tag="ident")
        warm_sb = consts.tile([P, QSUP], F16, tag="warm")

        xk_r = xtk.rearrange("(c p) s -> p c s", p=P)
        xq_r = xtq.rearrange("(c p) s -> p c s", p=P)
        xv_r = xtv.rearrange("(c p) s -> p c s", p=P)

        # ---- PE warmup: keep the tensor engine busy from ~0.5us so the
        # p-state ramp completes while the first DMAs are in flight.  The
        # warmup matmuls write the sc psum tag (first real use ~9.5us). ----
        nc.vector.memset(warm_sb[:], 0.0)
        # warmup matmuls: first at pstate-low (1.538 ns/col), then mid
        # (0.833) until 3us of continuous PE busy, then full (0.4167).
        # pe_cursor tracks the analytic PE completion time so later padding
        # can fill exactly until each DMA lands.
        # the Tile start barrier + memset + sem put the first PE exec ~1.85us
        pe_state = {"t": 1850.0, "t0": 1850.0}

        def warm_mm(cols):
            ramp = pe_state["t"] - pe_state["t0"]
            cyc = 1.538 if ramp <= 100 else (0.833 if ramp <= 3000 else 0.4167)
            ps = sc_pool.tile([P, 2, QSUP], F32, tag="sc", name="warm")
            nc.tensor.matmul(ps[:, 0, 0:cols], warm_sb[:, 0:P],
                             warm_sb[:, 0:cols], start=True, stop=True)
            pe_state["t"] += cols * cyc

        def pe_advance(ns, gate=None):
            # account a real-work emission on the analytic PE cursor
            if gate is not None:
                pe_state["t"] = max(pe_state["t"], gate)
            pe_state["t"] += ns

        def pad_until(t_target):
            while pe_state["t"] < t_target - 160.0:
                warm_mm(128)

        while pe_state["t"] < 4700.0:
            warm_mm(512)

        # ---- input DMAs, ordered to the consumption schedule (SP issues
        # back-to-back; transfers serialize on the DMA-engine pool at
        # ~360 B/ns; track analytic landing times so compute pieces are not
        # emitted before their data exists) ----
        dma_land = {}
        dma_state = {"hwdge": 1057.0, "busy": 1057.0}

        def track_dma(key, nbytes, elem_bytes=1024):
            st = dma_state
            st["hwdge"] += 625.0
            rate = 360.0 if elem_bytes >= 512 else 180.0
            st["busy"] = max(st["busy"], st["hwdge"] + 650.0) + nbytes / rate
            dma_land[key] = st["busy"] + 950.0

        def w_dma(sb, src_halves, h, key):
            nc.sync.dma_start(
                sb[:, h].rearrange("p c d -> p (c d)"), src_halves[h][:])
            track_dma(key, P * FC * P * 2, 2048)

        def x_dma(sb, src, ch, key):
            # one 256-s chunk: [P, FC, 256], 512B elements, 512KB
            sl = slice(ch * 256, (ch + 1) * 256)
            nc.sync.dma_start(sb[:, :, sl], src[:, :, sl])
            track_dma(key, P * FC * 256 * 2, 512)

        # biases transfer right after wk0: their sems must land before the
        # first qt/kt evacs (~8.4us) but their 625ns HWDGE issues must not
        # push back the critical wq0/xq0/xq1 transfers
        w_dma(wq_sb, wq_h, 0, "wq0")
        x_dma(xq_sb, xq_r, 0, "xq0")
        x_dma(xq_sb, xq_r, 1, "xq1")
        w_dma(wk_sb, wk_h, 0, "wk0")
        nc.sync.dma_start(bk_sb[:], bk.rearrange("(c p) o -> p c o", p=P))
        nc.sync.dma_start(bq_sb[:], bq.rearrange("(c p) o -> p c o", p=P))
        nc.sync.dma_start(bv_row[:], bvr[:])
        track_dma("b", 3 * 1024, 4)
        for ch in range(8):
            x_dma(xk_sb, xk_r, ch, f"xk{ch}")
        x_dma(xq_sb, xq_r, 2, "xq2")
        x_dma(xq_sb, xq_r, 3, "xq3")
        w_dma(wv_sb, wv_h, 0, "wv0")
        for ch in range(8):
            x_dma(xv_sb, xv_r, ch, f"xv{ch}")
        x_dma(xq_sb, xq_r, 4, "xq4")
        x_dma(xq_sb, xq_r, 5, "xq5")
        x_dma(xq_sb, xq_r, 6, "xq6")
        x_dma(xq_sb, xq_r, 7, "xq7")
        w_dma(wq_sb, wq_h, 1, "wq1")
        w_dma(wk_sb, wk_h, 1, "wk1")
        w_dma(wv_sb, wv_h, 1, "wv1")
        nc.sync.dma_start(wo_sb[:], wot.rearrange("(c p) n -> p c n", p=P))
        track_dma("wo", DC * D * 2, 2048)
        nc.sync.dma_start(ident_sb[:], ident[:])
        track_dma("ident", P * P * 2, 256)

        T_EXP0 = float(TEXP0)    # first-exp estimate for step-time mapping
        STEP_NS = float(STEPNS)

        def land_step(*keys):
            t = max(dma_land[k] for k in keys)
            return max(0, int((t - T_EXP0) / STEP_NS) + 1)

        # ---- persistent activations ----
        kt_sb = qkv.tile([P, 2, S], F16, tag="kt")   # [hh*64+d, pair, s]
        qt_sb = qkv.tile([P, 2, S], F16, tag="qt")
        v_sb = qkv.tile([P, KB, HPC, DK + 1], F16, tag="v")
        nc.vector.memset(v_sb[:, :, :, DK:], 1.0)
        outt_sb = qkv.tile([P, 2, S], F16, tag="outt")  # [(hh,d), pair, q]
        ones_f32 = consts.tile([1, P], F32, tag="ones_f32")
        nc.vector.memset(ones_f32[:], 1.0)
        bv_rep = consts.tile([P, DC], F32, tag="bvrep")

        # ---- piece-split projection chains on two alternating psum tags ----
        chain_ctr = [0]

        def next_tag():
            chain_ctr[0] += 1
            return f"ps{chain_ctr[0] % 2}"

        def qk_chain_pieces(x_sb, w_sb, b_sb, t_sb, dc, ss, sh,
                            pool=None, tag=None):
            """One 256-s half: 2 pieces x 4 matmuls (~430ns each), evac on
            the last."""
            tag = tag or next_tag()
            pl = pool or psp
            state = {}
            sl = slice(ss * 512 + sh * 256, ss * 512 + (sh + 1) * 256)

            def piece(i):
                if i == 0:
                    state["ps"] = pl.tile([P, 256], F32, tag=tag, name="ps")
                ps = state["ps"]
                for fc in range(4 * i, 4 * i + 4):
                    nc.tensor.matmul(
                        ps[:],
                        w_sb[:, dc, fc, :],
                        x_sb[:, fc, sl],
                        start=(fc == 0), stop=(fc == FC - 1),
                    )
                if i == 1:
                    nc.vector.tensor_scalar(
                        t_sb[:, dc, sl],
                        ps[:], b_sb[:, dc, :], None, mybir.AluOpType.add,
                    )
            return [lambda i=i: piece(i) for i in range(2)]

        def kcol_piece(dc, kb, pool=None, tag=None):
            """One ~430ns piece: full-contraction k-proj for one 128-col
            block of kt, so sc(kb) consumers get per-kb granularity."""
            def piece():
                ps = (pool or psp).tile([P, P], F32,
                                        tag=tag or next_tag(), name="ps")
                for fc in range(FC):
                    nc.tensor.matmul(
                        ps[:],
                        wk_sb[:, dc, fc, :],
                        xk_sb[:, fc, kb * P:(kb + 1) * P],
                        start=(fc == 0), stop=(fc == FC - 1),
                    )
                nc.vector.tensor_scalar(
                    kt_sb[:, dc, kb * P:(kb + 1) * P],
                    ps[:], bk_sb[:, dc, :], None, mybir.AluOpType.add,
                )
            return [piece]

        def v_chain_piece(sb, hhalf):
            """One ~430ns piece: 8 matmuls of N=128 for 2 heads + evac."""
            tag = next_tag()

            def piece():
                ps = psp.tile([P, P], F32, tag=tag, name="ps")
                for fc in range(FC):
                    nc.tensor.matmul(
                        ps[:],
                        xv_sb[:, fc, sb * P:(sb + 1) * P],
                        wv_sb[:, hhalf, fc, :],
                        start=(fc == 0), stop=(fc == FC - 1),
                    )
                nc.vector.tensor_tensor(
                    v_sb[:, sb, 2 * hhalf:2 * hhalf + 2, :DK],
                    ps[:].rearrange("p (h d) -> p h d", h=2),
                    bv_rep[:, hhalf * P:(hhalf + 1) * P]
                        .rearrange("p (h d) -> p h d", h=2),
                    mybir.AluOpType.add,
                )
            return [piece]

        def bv_piece():
            def piece():
                ps = psp.tile([P, DC], F32, tag=next_tag(), name="ps")
                nc.tensor.matmul(ps[:], ones_f32[:], bv_row[:],
                                 start=True, stop=True)
                nc.vector.tensor_copy(bv_rep[:], ps[:])
            return [piece]

        def oproj_piece(pool, sb_i, split_evac=False, split_dma=False,
                        alt=False, dma_separate=False):
            """One s-block of the O-projection as two ~430ns sub-pieces
            (one 512-col psum chain + evac each). split_evac puts one half's
            evacuation on the (tail-idle) ACT, alternating engines per sb
            when alt is set; split_dma issues one DMA per 512-col half so
            the last bytes leave earlier."""
            state = {}

            def evac(o_sb, half, on_act):
                if on_act:
                    nc.scalar.activation(
                        o_sb[:, half * 512:(half + 1) * 512], state["ps"],
                        mybir.ActivationFunctionType.Copy,
                        bias=0.0, scale=1.0,
                    )
                else:
                    nc.vector.tensor_copy(
                        o_sb[:, half * 512:(half + 1) * 512], state["ps"])

            def half_piece(half):
                if half == 0:
                    state["o"] = opool.tile([P, D], F16, tag="o", name="o_sb")
                o_sb = state["o"]
                ps = pool.tile([P, 512], F32, tag=next_tag(), name="ps")
                state["ps"] = ps[:]
                for pair in range(2):
                    nc.tensor.matmul(
                        ps[:],
                        outt_sb[:, pair, sb_i * P:(sb_i + 1) * P],
                        wo_sb[:, pair, half * 512:(half + 1) * 512],
                        start=(pair == 0), stop=(pair == 1),
                    )
                evac(o_sb, half,
                     split_evac and (half == 1) != (alt and sb_i % 2 == 1))
                # output DMAs issue from the DVE queue: they naturally order
                # after the DVE evacs and keep the SP queue free so the
                # outt DMA-transposes (which oproj ldweights wait on) are
                # never head-of-line blocked behind an output DMA's evac
                if split_dma:
                    nc.sync.dma_start(
                        outp[sb_i * P:(sb_i + 1) * P,
                             half * 512:(half + 1) * 512],
                        o_sb[:, half * 512:(half + 1) * 512])
                elif half == 1 and not dma_separate:
                    nc.sync.dma_start(
                        outp[sb_i * P:(sb_i + 1) * P, :], o_sb[:])
            pieces = [lambda h=h: half_piece(h) for h in range(2)]
            if dma_separate:
                # the output DMA as its own queue item, popped ~2 steps after
                # the evac piece, so the SP queue never stalls on an evac sem
                # with a norm's outt DMA-transpose queued behind it
                pieces.append(lambda: nc.sync.dma_start(
                    outp[sb_i * P:(sb_i + 1) * P, :], state["o"][:]))
            return pieces

        # ---- deadline-ordered work queue: (deadline, ready, piece) ----
        work = deque()

        def enq(deadline, pieces, ready=0, weight=1):
            for p in pieces:
                work.append((deadline, ready, p, weight))

        def drain(s):
            # Tile deps are trace-ordered: a piece MUST be emitted by its
            # deadline (one step before its consumer) no matter what;
            # ready only gates the opportunistic early pops.  weight-0 items
            # (bare SP DMAs) don't consume the per-step budget.
            n = 0
            budget = DRAIN_EARLY if s < 8 else 1
            while work:
                dl, rdy, fn, wt = work[0]
                if dl <= s + 1 or (n < budget and rdy <= s):
                    work.popleft()
                    fn()
                    n += wt
                else:
                    break

        # phase A: only what sc/exp of (qs0, pair0, kb0..1) needs, in DMA
        # arrival order, padded with warmup matmuls so the PE never idles
        # long enough to reset its p-state ramp; kcol 2/3 drain into the
        # first pipeline steps.  qk pieces borrow still-unused av-pool banks
        # so phase A doesn't serialize on the two projection-psum tags.
        pad_until(dma_land["xq0"])
        for fn in qk_chain_pieces(xq_sb, wq_sb, bq_sb, qt_sb, 0, 0, 0,
                                  pool=av_pool, tag="av"):
            fn()
        pe_advance(854, gate=dma_land["xq0"])
        pad_until(dma_land["xq1"])
        for fn in qk_chain_pieces(xq_sb, wq_sb, bq_sb, qt_sb, 0, 0, 1,
                                  pool=av_pool, tag="av"):
            fn()
        pe_advance(854, gate=dma_land["xq1"])
        pad_until(dma_land["xk0"])
        kcol_piece(0, 0)[0]()
        enq(0, kcol_piece(0, 1), ready=land_step("wk0", "xk0"))
        enq(1, kcol_piece(0, 2), ready=land_step("wk0", "xk1"))
        enq(2, kcol_piece(0, 3), ready=land_step("wk0", "xk1"))

        enq(2, bv_piece())
        for kb in range(4, KB):
            enq(max(0, kb - 2), kcol_piece(0, kb),
                ready=land_step("wk0", f"xk{kb // 2}"))
        for sb in range(KB):
            enq(sb + V0_DL, v_chain_piece(sb, 0),
                ready=land_step("wv0", f"xv{sb // 2}"))
        for ss in range(1, NQ):
            for sh in range(2):
                enq(16 * ss + QK_DL + 3 * sh,
                    qk_chain_pieces(xq_sb, wq_sb, bq_sb, qt_sb, 0, ss, sh),
                    ready=land_step("wq0", f"xq{2 * ss + sh}"))
        # pair-1 activations: first pair-1 block is qs3 at step 64
        for kb in range(KB):
            enq(K1_DL + kb, kcol_piece(1, kb),
                ready=land_step("wk1", f"xk{kb // 2}"))
        for i, qs in enumerate(q for q, pr in BLOCKS if pr == 1):
            for sh in range(2):
                enq(Q1_DL + 4 * i + 2 * sh,
                    qk_chain_pieces(xq_sb, wq_sb, bq_sb, qt_sb, 1, qs, sh),
                    ready=land_step("wq1", f"xq{2 * qs + sh}"))
        for sb in range(KB):
            # earliest pair-1 consumer is block 4 at step 64+sb+LAGS[4]
            enq(V1_DL + sb, v_chain_piece(sb, 1),
                ready=land_step("wv1", f"xv{sb // 2}"))
        # O-projection for qs 3/0/1 as soon as both pairs' transposes done
        for i in range(4):
            for qs in (3, 0, 1):
                rd = OPROJ_READY[qs]
                dma_sep = qs != 1   # qs1's pieces pop in the chase already
                pieces = oproj_piece(psp, 4 * qs + i, dma_separate=dma_sep)
                enq(min(rd + OP_SPREAD * i, NSTEP + 30), pieces[:2],
                    ready=rd)
                if dma_sep:
                    enq(rd + OP_SPREAD * i + 2, pieces[2:], ready=rd + 1,
                        weight=0)
        # sub-pieces of one sb must stay adjacent after the deadline sort
        # (they share an o_sb tile); deadlines above are unique per sb
        work = deque(sorted(work, key=lambda x: (x[0], -x[3])))

        # ---- the 128-step pipeline ----
        pt_tiles = {}
        av_tiles = {}

        def emit_sc_exp(s):
            qs, pair = BLOCKS[s // KB]
            kb = s % KB
            q0 = qs * QSUP
            sc_ps = sc_pool.tile([P, 2, QSUP], F32, tag="sc", name="sc")
            for hh in range(2):
                hp = hh * DK
                nc.tensor.matmul(
                    sc_ps[:, hh, :],
                    kt_sb[hp:hp + DK, pair, kb * P:(kb + 1) * P],
                    qt_sb[hp:hp + DK, pair, q0:q0 + QSUP],
                    start=True, stop=True,
                )
            pt = ptpool.tile([P, 2, QSUP], F16, tag="pt", name="pt")
            nc.scalar.activation(
                pt[:], sc_ps[:], mybir.ActivationFunctionType.Exp,
                bias=0.0, scale=0.125,
            )
            pt_tiles[s] = pt

        def emit_av(s):
            blk = s // KB
            qs, pair = BLOCKS[blk]
            kb = s % KB
            pt = pt_tiles.pop(s)
            if kb == 0:
                av_tiles[blk] = [
                    av_pool.tile([P, NQ, DK + 1], F32, tag="av", name=f"av{hh}")
                    for hh in range(2)
                ]
            av = av_tiles[blk]
            for hh in range(2):
                h = 2 * pair + hh
                for qt in range(NQ):
                    # one start=True per PSUM bank: the bank-granular
                    # pending-zero covers the other interleaved chains
                    nc.tensor.matmul(
                        av[hh][:, qt, :],
                        pt[:, hh, qt * P:(qt + 1) * P],
                        v_sb[:, kb, h, :],
                        start=(kb == 0 and qt == 0),
                        stop=(kb == KB - 1 and qt == NQ - 1),
                        skip_group_check=True,
                    )

        def emit_norm_transpose(blk, qt_range=None, skip_transpose=False):
            qs, pair = BLOCKS[blk]
            av = av_tiles[blk] if qt_range else av_tiles.pop(blk)
            att = attpool.tile([P, NQ, 2, DK], F16, tag="att", name="att")
            for hh in range(2):
                rec = dyn.tile([P, NQ, 1], F32, tag=f"rec{hh}", name="rec")
                nc.vector.reciprocal(rec[:, :, 0], av[hh][:, :, DK])
                nc.vector.tensor_tensor(
                    att[:, :, hh, :],
                    av[hh][:, :, :DK],
                    rec[:].broadcast_to([P, NQ, DK]),
                    mybir.AluOpType.mult,
                )
            if skip_transpose:
                return att
            # one batched xbar transpose for the whole 512-q block:
            # src [128 q, (qt d')] -> dst[d', qt, q], which is exactly the
            # outt column layout (qg = qs*4 + qt); 1 SP issue instead of 4
            nc.sync.dma_start_transpose(
                outt_sb[:, pair, qs * QSUP:(qs + 1) * QSUP]
                    .rearrange("p (t q) -> p t q", t=NQ),
                att[:, :, :, :],
            )

        def emit_block_step(b, sa):
            # each block's first av step is deferred one step (doubled at
            # kb==1): the previous block's norm (which frees the av psum
            # bank) then has a full extra step before the first-av matmul
            # would otherwise stall the PE on the DVE norm's completion
            kb = sa % KB
            if AV_KB0_DELAY:
                if kb == 0:
                    return
                if kb == 1:
                    emit_av(sa - 1)
                emit_av(sa)
            else:
                emit_av(sa)
            if kb == KB - 1 and b < NBLK - 1:
                # norm before drain: the next block's first-av matmul waits
                # the norm's DVE completion (av psum reuse), so the norm
                # must not queue behind a drained oproj evacuation
                emit_norm_transpose(b)

        for s in range(NSTEP + LAGS[-1]):
            if s < NSTEP:
                emit_sc_exp(s)
            for b in range(NBLK):
                sa = s - LAGS[b]
                if 0 <= sa < NSTEP and sa // KB == b:
                    emit_block_step(b, sa)
            drain(min(s, NSTEP))
        while work:
            work.popleft()[2]()

        # ---- tail: last block (qs2, pair1) normed per 128-q group, each
        # group immediately transposed on the (now-idle) PE via is_transpose
        # + identity and fed to its O-projection s-block on the freed psum
        # banks; evacs alternate DVE / idle ACT; the first sb's DMA is split
        # per half so the output stream starts as early as possible ----
        qs_l, pair_l = BLOCKS[NBLK - 1]
        av_l = av_tiles.pop(NBLK - 1)
        att_l = attpool.tile([P, NQ, 2, DK], F16, tag="att", name="att")
        for hh in range(2):
            rec = dyn.tile([P, NQ, 1], F32, tag=f"rec{hh}", name="rec")
            nc.vector.reciprocal(rec[:, :, 0], av_l[hh][:, :, DK])
            nc.vector.tensor_tensor(
                att_l[:, :, hh, :],
                av_l[hh][:, :, :DK],
                rec[:].broadcast_to([P, NQ, DK]),
                mybir.AluOpType.mult,
            )
        psp.release()
        av_pool.release()
        sc_pool.release()
        op2 = tc.alloc_tile_pool(name="op2", bufs=2, space="PSUM")
        for qt in range(NQ):
            tr_ps = op2.tile([P, P], F16, tag="tr", bufs=4, name="tr")
            nc.tensor.matmul(tr_ps[:], att_l[:, qt, :, :], ident_sb[:],
                             is_transpose=True, start=True, stop=True)
            qg = qs_l * NQ + qt
            nc.vector.tensor_copy(
                outt_sb[:, pair_l, qg * P:(qg + 1) * P], tr_ps[:])
        for qt in range(NQ):
            for fn in oproj_piece(op2, 4 * qs_l + qt, split_evac=True,
                                  split_dma=(qt == 0), alt=True):
                fn()

        for pool in (op2, opool, dyn, attpool, ptpool, qkv, consts):
            pool.release()

    split_multi_waits(nc)
    return nc


_NC_CACHE = None


def _pack_w_half(w_slice):
    """[128 dc, 1024 D] weight chunk -> partition-major [128 p, (fc d)]."""
    return np.ascontiguousarray(
        w_slice.T.reshape(FC, P, P).transpose(1, 0, 2).reshape(P, FC * P)
    )


def prep_in_maps(Q, K, V, W_q, b_q, W_k, b_k, W_v, b_v, W_o, b_o):
    """Host-side sharding: per-core input dicts (transposed, fp16-cast)."""
    f16 = np.float16
    Q, K, V = np.asarray(Q), np.asarray(K), np.asarray(V)
    xt = {}
    for b in range(B):
        xt[b] = (
            np.ascontiguousarray(Q[b].T).astype(f16),
            np.ascontiguousarray(K[b].T).astype(f16),
            np.ascontiguousarray(V[b].T).astype(f16),
        )
    in_maps = []
    for c in range(N_CORES):
        b = c // 4
        g = c % 4
        sl = slice(g * DC, (g + 1) * DC)
        wq_c = np.asarray(W_q)[sl, :].astype(f16)
        wk_c = np.asarray(W_k)[sl, :].astype(f16)
        wv_c = np.asarray(W_v)[sl, :].astype(f16)
        m = {
            "xtq": xt[b][0], "xtk": xt[b][1], "xtv": xt[b][2],
            "wot": np.ascontiguousarray(np.asarray(W_o)[:, sl].T).astype(f16),
            "bq": np.asarray(b_q)[sl].reshape(DC, 1).astype(np.float32),
            "bk": np.asarray(b_k)[sl].reshape(DC, 1).astype(np.float32),
            "bvr": np.asarray(b_v)[sl].reshape(1, DC).astype(np.float32),
            "ident": np.eye(P, dtype=f16),
        }
        for h in range(2):
            hs = slice(h * P, (h + 1) * P)
            m[f"wqt{h}"] = _pack_w_half(wq_c[hs])
            m[f"wkt{h}"] = _pack_w_half(wk_c[hs])
            m[f"wvt{h}"] = _pack_w_half(wv_c[hs])
        in_maps.append(m)
    return in_maps


def gather_out(partials, b_o):
    """Host-side unshard: sum the four W_o-row partials per batch + b_o."""
    out = np.zeros((B, S, D), np.float32)
    for c in range(N_CORES):
        out[c // 4] += np.asarray(partials[c]).astype(np.float32)
    out += np.asarray(b_o).astype(np.float32)
    return out


def kernel(Q, K, V, W_q, b_q, W_k, b_k, W_v, b_v, W_o, b_o):
    global _NC_CACHE
    in_maps = prep_in_maps(Q, K, V, W_q, b_q, W_k, b_k, W_v, b_v, W_o, b_o)
    if _NC_CACHE is None:
        _NC_CACHE = build_bass()
    res = run_bass_kernel_spmd(_NC_CACHE, in_maps, core_ids=list(range(N_CORES)))
    return gather_out([res.results[c]["outp"] for c in range(N_CORES)], b_o)
